# revision 12
# baseline (speedup 1.0000x reference)
"""Trainium2 Bass kernel for nn_GrassmannNN.

Math: the reference's Z2-graded (Grassmann) network collapses per-sample to a
chain of 32x32 matmuls selected by the sample's bits.  For each body layer m
(embedding e0/e1, core G = body_w[m]):
  bit=0:  x <- tanh(x @ M0_m),  M0_m = (sum_{j<16} e0_j G[:,j,:]) * blockdiag
  bit=1:  x <- tanh(x @ M1_m),  M1_m = (sum_{j>=16} e1_j G[:,j,:]) * antidiag/sign
Head: x0 built from embedding[0] by bit0, then tanh(x0 @ (head_w * blockdiag)).

Only 256 distinct bit patterns exist.  The pattern table is built by PREFIX
DOUBLING: S_1 (32,2) -> ... -> S_7 (32,128), where
  S_{i+1}[:, 0:2^i]       = M0_i.T @ S_i        (bit_i = 0)
  S_{i+1}[:, 2^i:2^{i+1}] = M1_i.T @ S_i        (bit_i = 1)
so total table matmul columns are 2+4+...+256 instead of 8*256.  The final
step is emitted transposed (patterns on partitions) giving the (256, 64)
output table directly.  Transition matrices for all 7 layers come from ONE
fused pair of fp32 matmuls (K = (layer, j) blocks of 128/96) followed by a
masked psum->SBUF copy and a reshaping SBUF->SBUF DMA.  The network is
chaotic (~1e4 error amplification through the 8 tanh layers), so the whole
table build stays fp32; only the final table values are cast to bf16 (a
last-stage, unamplified 0.4% rounding) so the one-hot gather matmuls run in
1-pass bf16.  A short burst of dummy bf16 matmuls at kernel start warms the
PE HAM clock gate (1.2 -> 2.4 GHz) while input DMAs stream.

Sharding: pure data parallel over the batch; each of the 8 cores computes the
(tiny) table redundantly and gathers its 1024-sample slice.
"""

import numpy as np
from contextlib import ExitStack

import concourse.bass as bass
import concourse.bacc as bacc
import concourse.tile as tile
import concourse.mybir as mybir
from concourse.bass_utils import run_bass_kernel_spmd

F32 = mybir.dt.float32
BF16 = mybir.dt.bfloat16
AF = mybir.ActivationFunctionType
OP = mybir.AluOpType

NCORES = 8
B = 8192
BC = B // NCORES          # 1024 samples per core
N = 256                   # distinct bit patterns
NWARM = 10                # dummy matmuls to warm the PE clock gate


def _host_consts():
    pd = np.concatenate([np.zeros(16, np.int64), np.ones(16, np.int64)])
    maskbd = ((pd[:, None] ^ pd[None, :]) == 0).astype(np.float32)
    maskads = (((pd[:, None] ^ pd[None, :]) == 1).astype(np.float32)
               * (1.0 - 2.0 * pd)[:, None].astype(np.float32))
    blob = np.zeros((128, 178), np.float32)
    # emask0 cols 0:8 : rows p=m*32+j (m=0..3 -> sites 1..4), col m*2 + (j>=16)
    for m in range(4):
        for j in range(32):
            blob[m * 32 + j, m * 2 + (j // 16)] = 1.0
    # emask1 cols 8:14 : rows p=m*32+j (m=0..2 -> sites 5..7)
    for m in range(3):
        for j in range(32):
            blob[m * 32 + j, 8 + m * 2 + (j // 16)] = 1.0
    blob[0:16, 14] = 1.0          # elh2 col 0
    blob[16:32, 15] = 1.0         # elh2 col 1
    blob[0:32, 16:48] = maskbd
    blob[0:128, 48] = np.arange(128, dtype=np.float32)
    blob[0:128, 49] = np.arange(128, 256, dtype=np.float32)
    blob[0:8, 50:178] = (2.0 ** np.arange(8, dtype=np.float32))[:, None]
    mrep = np.zeros((14, 1024), np.float32)
    mbd, mads = maskbd.reshape(-1), maskads.reshape(-1)
    for m in range(4):
        mrep[m * 2 + 0] = mbd
        mrep[m * 2 + 1] = mads
    for m in range(3):
        mrep[8 + m * 2 + 0] = mbd
        mrep[8 + m * 2 + 1] = mads
    return blob, mrep


def _emit(ctx: ExitStack, tc, t):
    nc = tc.nc
    cpool = ctx.enter_context(tc.tile_pool(name="consts", bufs=1))
    mpool = ctx.enter_context(tc.tile_pool(name="mats", bufs=1))
    spool = ctx.enter_context(tc.tile_pool(name="state", bufs=2))
    psE = ctx.enter_context(tc.tile_pool(name="psE", bufs=1, space="PSUM"))
    psS = ctx.enter_context(tc.tile_pool(name="psS", bufs=1, space="PSUM"))
    psF = ctx.enter_context(tc.tile_pool(name="psF", bufs=1, space="PSUM"))
    psG = ctx.enter_context(tc.tile_pool(name="psG", bufs=1, space="PSUM"))

    # ---- PE warm-up: dummy bf16 matmuls while DMAs stream ----
    pg = psG.tile([128, 512], F32, tag="pg")
    wtile = cpool.tile([128, 512], BF16)
    nc.vector.memset(wtile[:], 0.0)
    for _ in range(NWARM):
        nc.tensor.matmul(pg[:], wtile[:, 0:128], wtile[:], start=True, stop=True)

    # ---- input DMAs (split across the two HWDGE queues) ----
    tBall0 = cpool.tile([128, 1024], F32)
    nc.sync.dma_start(tBall0[:], t["ball0"].ap())
    tBall1 = cpool.tile([96, 1024], F32)
    nc.sync.dma_start(tBall1[:], t["ball1"].ap())
    tBlob = cpool.tile([128, 178], F32)
    nc.scalar.dma_start(tBlob[:], t["blob"].ap())
    tMrep0 = cpool.tile([8, 1024], F32)
    nc.scalar.dma_start(tMrep0[:], t["mrep"].ap()[0:8, :])
    tMrep1 = cpool.tile([6, 1024], F32)
    nc.scalar.dma_start(tMrep1[:], t["mrep"].ap()[8:14, :])
    tDat = cpool.tile([8, BC], BF16)
    nc.scalar.dma_start(tDat[:], t["dataT"].ap())
    tEv0 = cpool.tile([128, 1], F32)
    nc.scalar.dma_start(tEv0[:], bass.AP(t["emb"], 32, [[1, 128], [1, 1]]))
    tEv1 = cpool.tile([96, 1], F32)
    nc.scalar.dma_start(tEv1[:], bass.AP(t["emb"], 160, [[1, 96], [1, 1]]))
    tEvH = cpool.tile([32, 1], F32)
    nc.scalar.dma_start(tEvH[:], bass.AP(t["emb"], 0, [[1, 32], [1, 1]]))
    tHead = cpool.tile([32, 32], F32)
    nc.scalar.dma_start(tHead[:], t["head"].ap())

    # ---- small operand builds (DVE) ----
    tEbd0 = mpool.tile([128, 8], F32)
    nc.vector.tensor_scalar(tEbd0[:], tBlob[:, 0:8], tEv0[:], None, OP.mult)
    tEbd1 = mpool.tile([96, 6], F32)
    nc.vector.tensor_scalar(tEbd1[:], tBlob[0:96, 8:14],
                            tEv1[:], None, OP.mult)
    tMh = mpool.tile([32, 32], F32)
    nc.vector.tensor_mul(tMh[:], tHead[:], tBlob[0:32, 16:48])
    tX0 = mpool.tile([32, 2], F32)
    nc.vector.tensor_scalar(tX0[:], tBlob[0:32, 14:16],
                            tEvH[:], None, OP.mult)
    tPow = mpool.tile([8, 128], BF16)
    nc.vector.tensor_copy(tPow[:], tBlob[0:8, 50:178])

    # ---- fused E-contraction: all 7 layers' (M0, M1) in two K-blocks ----
    tMall = mpool.tile([32, 448], F32)
    pe0 = psE.tile([8, 1024], F32, tag="pe0")
    nc.tensor.matmul(pe0[:, 0:512], tEbd0[:], tBall0[:, 0:512],
                     start=True, stop=True)
    nc.tensor.matmul(pe0[:, 512:1024], tEbd0[:], tBall0[:, 512:1024],
                     start=True, stop=True)
    dpool = ctx.enter_context(tc.tile_pool(name="dram", bufs=1, space="DRAM"))
    tWm0 = mpool.tile([8, 1024], F32)
    nc.vector.tensor_mul(tWm0[:], pe0[:], tMrep0[:])
    dWm0 = dpool.tile([8, 1024], F32)
    nc.sync.dma_start(dWm0[:], tWm0[:])
    # dst[i, lb*32+k] = Wm0[lb, i*32+k]; DRAM-side 3D AP, plain SBUF dst
    nc.sync.dma_start(tMall[:, 0:256],
                      bass.AP(dWm0.tensor, 0, [[32, 32], [1024, 8], [1, 32]]))
    pe1 = psE.tile([6, 1024], F32, tag="pe1")
    nc.tensor.matmul(pe1[:, 0:512], tEbd1[:], tBall1[:, 0:512],
                     start=True, stop=True)
    nc.tensor.matmul(pe1[:, 512:1024], tEbd1[:], tBall1[:, 512:1024],
                     start=True, stop=True)
    tWm1 = mpool.tile([6, 1024], F32)
    nc.vector.tensor_mul(tWm1[:], pe1[:], tMrep1[:])
    dWm1 = dpool.tile([6, 1024], F32)
    nc.scalar.dma_start(dWm1[:], tWm1[:])
    nc.scalar.dma_start(tMall[:, 256:448],
                        bass.AP(dWm1.tensor, 0, [[32, 32], [1024, 6], [1, 32]]))

    def mslice(m, b):
        off = (m * 2 + b) * 32 if m < 4 else 256 + ((m - 4) * 2 + b) * 32
        return tMall[:, off:off + 32]

    # ---- head: S_1 (32, 2) ----
    ps1 = psS.tile([32, 2], F32, tag="ps")
    nc.tensor.matmul(ps1[:], tMh[:], tX0[:], start=True, stop=True)
    S = spool.tile([32, 2], F32, tag="S1")
    nc.scalar.activation(S[:], ps1[:], AF.Tanh)

    # ---- sample index -> one-hot (overlaps table build on PE/DVE) ----
    tOh0 = cpool.tile([128, BC], BF16)
    tOh1 = cpool.tile([128, BC], BF16)
    for h in range(2):
        # reuse the gather psum bank for the index matmuls
        nc.tensor.matmul(pg[:], tPow[:], tDat[:, h * 512:(h + 1) * 512],
                         start=True, stop=True)
        nc.vector.tensor_scalar(tOh0[:, h * 512:(h + 1) * 512], pg[:],
                                tBlob[:, 48:49], None, OP.is_equal)
        nc.vector.tensor_scalar(tOh1[:, h * 512:(h + 1) * 512], pg[:],
                                tBlob[:, 49:50], None, OP.is_equal)

    # ---- prefix-doubling rollout: S_i (32, 2^i) -> S_7 (32, 128) ----
    for i in range(1, 7):
        w = 1 << i
        ps = psS.tile([32, 2 * w], F32, tag="ps")
        nc.tensor.matmul(ps[:, 0:w], mslice(i - 1, 0), S[:],
                         start=True, stop=True)
        nc.tensor.matmul(ps[:, w:2 * w], mslice(i - 1, 1), S[:],
                         start=True, stop=True)
        S2 = spool.tile([32, 2 * w], F32, tag=f"S{i + 1}")
        nc.scalar.activation(S2[:], ps[:], AF.Tanh)
        S = S2

    # ---- final layer transposed: bf16 table halves (128 patterns, 64) ----
    Ths = []
    for b in range(2):
        pf = psF.tile([128, 32], F32, tag=f"pf{b}")
        nc.tensor.matmul(pf[:], S[:], mslice(6, b), start=True, stop=True)
        T = mpool.tile([128, 64], BF16, tag=f"T{b}")
        nc.gpsimd.memset(T[:], 0.0)
        nc.scalar.activation(T[:, 0:16], pf[:, 0:16], AF.Tanh)
        nc.scalar.activation(T[:, 48:64], pf[:, 16:32], AF.Tanh)
        Ths.append(T)

    # ---- gather: one-hot bf16 matmuls into one psum bank ----
    for bt in range(8):
        nc.tensor.matmul(pg[:, bt * 64:(bt + 1) * 64],
                         tOh0[:, bt * 128:(bt + 1) * 128], Ths[0][:],
                         start=True, stop=False)
        nc.tensor.matmul(pg[:, bt * 64:(bt + 1) * 64],
                         tOh1[:, bt * 128:(bt + 1) * 128], Ths[1][:],
                         start=False, stop=True)

    # ---- stage + write out (both HWDGE queues) ----
    for h in range(2):
        og = mpool.tile([128, 256], F32, tag=f"og{h}")
        nc.vector.tensor_copy(og[:], pg[:, h * 256:(h + 1) * 256])
        eng = nc.sync if h == 0 else nc.scalar
        eng.dma_start(bass.AP(t["out"], h * 32768, [[64, 128], [8192, 4], [1, 64]]),
                      og[:].rearrange("p (b c) -> p b c", b=4))


def build_program():
    nc = bacc.Bacc("TRN2", target_bir_lowering=False, debug=False,
                   enable_asserts=False, num_devices=NCORES)
    t = {}
    t["ball0"] = nc.dram_tensor("ball0", [128, 1024], F32, kind="ExternalInput")
    t["ball1"] = nc.dram_tensor("ball1", [96, 1024], F32, kind="ExternalInput")
    t["blob"] = nc.dram_tensor("blob", [128, 178], F32, kind="ExternalInput")
    t["mrep"] = nc.dram_tensor("mrep", [14, 1024], F32, kind="ExternalInput")
    t["dataT"] = nc.dram_tensor("dataT", [8, BC], BF16, kind="ExternalInput")
    t["emb"] = nc.dram_tensor("emb", [256], F32, kind="ExternalInput")
    t["head"] = nc.dram_tensor("head", [32, 32], F32, kind="ExternalInput")
    t["out"] = nc.dram_tensor("out", [BC, 64], F32, kind="ExternalOutput")
    with tile.TileContext(nc) as tc:
        with ExitStack() as ctx:
            _emit(ctx, tc, t)
    nc.compile()
    return nc


def make_in_maps(data, embedding, head_w, body_w):
    import ml_dtypes
    bf = ml_dtypes.bfloat16
    data = np.asarray(data)
    if data.dtype == np.int64:
        d32 = data.view(np.int32).reshape(B, 16)[:, ::2]
    else:
        d32 = data.astype(np.int32, copy=False)
    blob, mrep = _host_consts()
    ballf = np.ascontiguousarray(
        np.asarray(body_w, np.float32).transpose(0, 2, 1, 3)).reshape(224, 1024)
    base = {
        "ball0": np.ascontiguousarray(ballf[0:128]),
        "ball1": np.ascontiguousarray(ballf[128:224]),
        "blob": blob,
        "mrep": mrep,
        "emb": np.ascontiguousarray(embedding, np.float32).reshape(-1),
        "head": np.ascontiguousarray(head_w, np.float32),
    }
    in_maps = []
    for c in range(NCORES):
        dslice = np.ascontiguousarray(
            d32[c * BC:(c + 1) * BC].T).astype(np.float32).astype(bf)
        in_maps.append({**base, "dataT": dslice})
    return in_maps


_CACHE = {}


def kernel(data, embedding, head_w, body_w, **kw):
    nc = _CACHE.get("nc")
    if nc is None:
        nc = build_program()
        _CACHE["nc"] = nc
    in_maps = make_in_maps(data, embedding, head_w, body_w)
    res = run_bass_kernel_spmd(nc, in_maps, core_ids=list(range(NCORES)))
    out = np.concatenate([res.results[c]["out"] for c in range(NCORES)], axis=0)
    return out.reshape(B, 2, 32)
